# revision 28
# baseline (speedup 1.0000x reference)
"""BitTransformerEncoderLayer on 8 TRN2 NeuronCores.

Strategy: pure data parallelism over batch (B=8 == n_cores). Each core runs the
full layer for one batch element; no collectives.

Precision plan (rel-err budget 2e-2; simulated 9.1e-3):
- in_proj / out_proj / attn@v run as fp8 e4m3 DoubleRow matmuls (2 fp8 weights
  per PE cell, ~1.4-2x bf16 throughput). Weights absmax-prescaled into fp8
  range on host; descale folded into PSUM eviction.
- softmax denominator comes from a DoubleRow ones-matmul whose [P,512] output
  is already broadcast across partitions; divide via fast-approx reciprocal.
- scores matmul stays bf16 (contraction 128 cannot double-pump).
- BitLinear ff1/ff2 matmuls stay exact: ternary weights (bf16) x integer
  activations (bf16), fp32 PSUM. ff1 activations are prescaled by the c1
  dequant scale (bf16 rounding only), so ff1's PSUM evictions go straight to
  ACT with per-partition scale/bias; snake sin^2 uses sin(2x+pi/2)=cos(2x).

kernel(**inputs) takes the FULL unsharded inputs and returns the FULL output.
"""

import numpy as np

P = 128
EPS = 1e-8
MAGIC = 12582912.0  # 1.5 * 2**23: fp32 add/sub rounds to nearest-even integer
NCORES = 8
EXP_BIAS = float(-np.log(4.0))  # keeps exp(scores) < 240 for e4m3

# Problem dims (hardcoded per spec)
B_FULL, S_FULL, D_FULL, H_FULL, DFF_FULL = 8, 1024, 2048, 16, 8192

_CACHE = {}


# ---------------------------------------------------------------- host prep

def _quant_w(w):
    scale = np.maximum(np.mean(np.abs(w), dtype=np.float32), np.float32(1e-5))
    q = np.clip(np.round(w / scale), -1.0, 1.0).astype(np.float32)
    return q, float(scale)


def _lhsT_blocks(w):
    """w [M, K] -> [M/P, P(k), K/P, P(m)]; [mo, :, ko, :] = w-block(mo, ko).T"""
    M, K = w.shape
    t = w.reshape(M // P, P, K // P, P)  # [mo, pm, ko, pk]
    return np.ascontiguousarray(t.transpose(0, 3, 2, 1))


def _lhsT_blocks_dr(w):
    """w [M, K] -> [M/P, P(pk), K/256, 2, P(pm)] for DoubleRow stationary.

    [mo, pk, ko2, i, pm] = w[mo*P+pm, ko2*256 + i*128 + pk]
    """
    M, K = w.shape
    t = w.reshape(M // P, P, K // 256, 2, P)  # [mo, pm, ko2, i, pk]
    return np.ascontiguousarray(t.transpose(0, 4, 2, 3, 1))


def _rhs_chunks(w, nch):
    """w [N, K] -> [N/nch, K/P, P, nch]; [no, ko, p, j] = w[no*nch+j, ko*P+p]"""
    N, K = w.shape
    t = w.reshape(N // nch, nch, K // P, P)  # [no, j, ko, p]
    return np.ascontiguousarray(t.transpose(0, 2, 3, 1))


def _rhs_chunks_dr(w, nch):
    """w [N, K] -> [N/nch, K/256, P, 2, nch] for DoubleRow moving.

    [no, ko2, p, i, j] = w[no*nch+j, ko2*256 + i*128 + p]
    """
    N, K = w.shape
    t = w.reshape(N // nch, nch, K // 256, 2, P)  # [no, j, ko2, i, p]
    return np.ascontiguousarray(t.transpose(0, 2, 4, 3, 1))


def _per_part(v):
    """[M] -> [P, M/P]; out[p, mo] = v[mo*P + p]"""
    return np.ascontiguousarray(v.reshape(-1, P).T)


def _bcast_row(v):
    return np.ascontiguousarray(np.broadcast_to(v[None, :], (P, v.shape[0])))


def _prep_arrays(inputs, S, D, H, DFF):
    import ml_dtypes

    bf16 = ml_dtypes.bfloat16
    f8 = ml_dtypes.float8_e4m3
    f32 = np.float32
    g = lambda k: np.asarray(inputs[k], dtype=f32)

    w1q, ws1 = _quant_w(g("ff1_w"))   # [DFF, D]
    w2q, ws2 = _quant_w(g("ff2_w"))   # [D, DFF]
    ncd = min(512, D)

    w_in = g("in_proj_w")
    sw_in = float(240.0 / max(np.abs(w_in).max(), 1e-20))
    wo = g("out_proj_w")
    swo = float(240.0 / max(np.abs(wo).max(), 1e-20))

    alpha = g("alpha")
    b1 = g("ff1_b")
    gam = (1.0 / (g("beta") + np.float32(1e-9))).astype(f32)

    def to_f8(x):
        return np.clip(x, -240.0, 240.0).astype(f8)

    arrays = {
        "w_in_dr": to_f8(_lhsT_blocks_dr(w_in * sw_in)),     # [3D/P, P, D/256, 2, P]
        "wo_dr": to_f8(_rhs_chunks_dr(wo * swo, ncd)),       # [D/ncd, D/256, P, 2, ncd]
        "w1_blk": _lhsT_blocks(w1q).astype(bf16),            # [DFF/P, P, D/P, P]
        "w2_chunk": _rhs_chunks(w2q, ncd).astype(bf16),      # [D/ncd, DFF/P, P, ncd]
        "bias_in": _per_part(g("in_proj_b")).astype(f32),    # [P, 3D/P]
        "a1_t": _per_part(alpha.astype(f32)),                # [P, DFF/P]
        "ab1_t": _per_part((alpha * b1).astype(f32)),
        "b1_t": _per_part(b1.astype(f32)),
        "gam_t": _per_part(gam.astype(f32)),
        "n1w_bc": _bcast_row(g("norm1_w")).astype(f32),      # [P, D]
        "n2w_bc": _bcast_row(g("norm2_w")).astype(f32),
        "b2_bc": _bcast_row(g("ff2_b")).astype(f32),
    }
    return arrays, ws1, ws2, sw_in, swo


# ---------------------------------------------------------------- device program

def build_program(nc, *, S, D, H, DFF, ws1, ws2, sw_in, swo):
    import concourse.mybir as mybir
    import concourse.tile as tile
    from concourse.bass import ts
    from concourse.masks import make_identity

    dt = mybir.dt
    AF = mybir.ActivationFunctionType
    OP = mybir.AluOpType
    DR = mybir.MatmulPerfMode.DoubleRow

    DH = D // H
    assert DH == P, "layout assumes head dim == 128"
    ST = S // P           # token tiles
    KD = D // P           # D contraction tiles
    KD2 = D // 256        # D contraction pair-tiles (DoubleRow)
    KF = DFF // P         # DFF contraction tiles / ff1 out tiles
    ST2 = ST // 2         # token pair-tiles (DoubleRow over keys)
    NCD = min(512, D)     # fo chunk for out_proj/ff2 (psum-bank sized)
    NOD = D // NCD
    NCS = min(512, S)     # s chunk
    NOS = S // NCS
    inv_sqrt_dh = float(1.0 / np.sqrt(DH))

    # ---- DRAM I/O ----
    src_d = nc.dram_tensor("src", [S, D], dt.float32, kind="ExternalInput")
    srcb_d = nc.dram_tensor("srcb", [S, D], dt.float32, kind="ExternalInput")  # src + out_proj_b
    w_in_d = nc.dram_tensor("w_in_dr", [3 * KD, P, KD2, 2, P], dt.float8e4, kind="ExternalInput")
    wo_d = nc.dram_tensor("wo_dr", [NOD, KD2, P, 2, NCD], dt.float8e4, kind="ExternalInput")
    w1_d = nc.dram_tensor("w1_blk", [KF, P, KD, P], dt.bfloat16, kind="ExternalInput")
    w2_d = nc.dram_tensor("w2_chunk", [NOD, KF, P, NCD], dt.bfloat16, kind="ExternalInput")
    bin_d = nc.dram_tensor("bias_in", [P, 3 * KD], dt.float32, kind="ExternalInput")
    a1_d = nc.dram_tensor("a1_t", [P, KF], dt.float32, kind="ExternalInput")
    ab1_d = nc.dram_tensor("ab1_t", [P, KF], dt.float32, kind="ExternalInput")
    b1_d = nc.dram_tensor("b1_t", [P, KF], dt.float32, kind="ExternalInput")
    gam_d = nc.dram_tensor("gam_t", [P, KF], dt.float32, kind="ExternalInput")
    n1w_d = nc.dram_tensor("n1w_bc", [P, D], dt.float32, kind="ExternalInput")
    n2w_d = nc.dram_tensor("n2w_bc", [P, D], dt.float32, kind="ExternalInput")
    b2_d = nc.dram_tensor("b2_bc", [P, D], dt.float32, kind="ExternalInput")
    out_d = nc.dram_tensor("out", [S, D], dt.float32, kind="ExternalOutput")
    # internal DRAM spills
    xb2_d = nc.dram_tensor("xb2_spill", [ST, P, D], dt.float32)  # x + b2
    h2_d = nc.dram_tensor("h2_spill", [KF, P, S], dt.float32)

    with tile.TileContext(nc) as tc:
        # ---------- persistent constants (whole kernel) ----------
        cp = tc.alloc_tile_pool(name="consts", bufs=1)
        ident = cp.tile([P, P], dt.bfloat16)
        make_identity(nc, ident)
        identf = cp.tile([P, P], dt.float32)
        make_identity(nc, identf)
        ones_dr = cp.tile([P, 2, P], dt.float8e4)
        nc.any.memset(ones_dr[:], 1.0)
        ones_1 = cp.tile([1, P], dt.float32)
        nc.any.memset(ones_1[:], 1.0)
        expb_sb = cp.tile([P, 1], dt.float32)
        nc.any.memset(expb_sb[:], EXP_BIAS)
        magicp = cp.tile([P, 1], dt.float32)
        nc.any.memset(magicp[:], MAGIC)
        magicn = cp.tile([P, 1], dt.float32)
        nc.any.memset(magicn[:], -MAGIC)
        bin_sb = cp.tile([P, 3 * KD], dt.float32)
        nc.sync.dma_start(out=bin_sb[:], in_=bin_d.ap())
        a1_sb = cp.tile([P, KF], dt.float32)
        nc.sync.dma_start(out=a1_sb[:], in_=a1_d.ap())
        ab1_sb = cp.tile([P, KF], dt.float32)
        nc.sync.dma_start(out=ab1_sb[:], in_=ab1_d.ap())
        b1_sb = cp.tile([P, KF], dt.float32)
        nc.sync.dma_start(out=b1_sb[:], in_=b1_d.ap())
        gam_sb = cp.tile([P, KF], dt.float32)
        nc.sync.dma_start(out=gam_sb[:], in_=gam_d.ap())
        c2_tok = cp.tile([P, ST], dt.float32)   # filled in phase 4
        sc2_bc = cp.tile([P, S], dt.float32)    # filled in phase 4

        # ================= phase 1: rmsnorm1 + transpose =================
        xp = tc.alloc_tile_pool(name="x2T_pool", bufs=1)
        x2T = xp.tile([P, KD, S], dt.float8e4)

        # in_proj weight pool allocated early so head 0's weights prefetch
        # on the sync queue ahead of phase 1's src loads
        p2w = tc.alloc_tile_pool(name="p2w", bufs=2)
        wblk0 = []
        for j, mo in ((0, 0), (1, KD), (2, 2 * KD)):
            wb = p2w.tile([P, KD2, 2, P], dt.float8e4, tag="wblk", bufs=3,
                          name=f"wblk0_{j}")
            nc.sync.dma_start(out=wb[:], in_=w_in_d.ap()[mo])
            wblk0.append(wb)

        p1 = tc.alloc_tile_pool(name="p1", bufs=2)
        p1c = tc.alloc_tile_pool(name="p1c", bufs=1)
        p1ps = tc.alloc_tile_pool(name="p1ps", bufs=4, space="PSUM")
        n1w_sb = p1c.tile([P, D], dt.float32)
        nc.sync.dma_start(out=n1w_sb[:], in_=n1w_d.ap())
        for mt in range(ST):
            xt = p1.tile([P, D], dt.float32, tag="xt", bufs=3)
            nc.sync.dma_start(out=xt[:], in_=src_d.ap()[ts(mt, P), :])
            sq = p1.tile([P, D], dt.float32, tag="sq", bufs=3)
            ss = p1.tile([P, 1], dt.float32, tag="ss")
            nc.scalar.activation(sq[:], xt[:], AF.Square, accum_out=ss[:])
            ms = p1.tile([P, 1], dt.float32, tag="ms")
            nc.vector.tensor_scalar(ms[:], ss[:], 1.0 / D, EPS, op0=OP.mult, op1=OP.add)
            rt = p1.tile([P, 1], dt.float32, tag="rt")
            nc.scalar.activation(rt[:], ms[:], AF.Sqrt)
            rs = p1.tile([P, 1], dt.float32, tag="rs")
            nc.vector.reciprocal(rs[:], rt[:])
            x2 = p1.tile([P, D], dt.bfloat16, tag="x2", bufs=3)
            nc.vector.scalar_tensor_tensor(x2[:], xt[:], rs[:], n1w_sb[:], op0=OP.mult, op1=OP.mult)
            for ko in range(KD):
                pt = p1ps.tile([P, P], dt.bfloat16, tag="tp")
                nc.tensor.transpose(pt[:], x2[:, ts(ko, P)], ident[:])
                nc.vector.tensor_copy(x2T[:, ko, ts(mt, P)], pt[:])
        p1ps.release()
        p1c.release()
        p1.release()

        # ================= phase 2: fused in_proj + attention =================
        # oT spans phases 2-3a, overlapping x2T's [1-2] — opposite allocator side
        op_ = tc.alloc_tile_pool(name="oT_pool", bufs=1, side="right")
        oT_all = op_.tile([P, KD, S], dt.float8e4)

        p2 = tc.alloc_tile_pool(name="p2", bufs=2)
        p2a = tc.alloc_tile_pool(name="p2a", bufs=2, space="PSUM")
        p2b = tc.alloc_tile_pool(name="p2b", bufs=2, space="PSUM")
        p2c = tc.alloc_tile_pool(name="p2c", bufs=1, space="PSUM")

        def attn_tail(h, expT, vT):
            # DoubleRow ones-matmul gives the softmax denominator already
            # broadcast across all 128 partitions; fast-approx reciprocal,
            # then o^T = (sum_t v^T @ exp^T) * rden folded into eviction.
            # Emitted one head late so the PE never waits on ACT's exp.
            for sc in range(NOS):
                pden = p2c.tile([P, NCS], dt.float32, tag="den", name=f"pden_{h}_{sc}")
                for t2 in range(ST2):
                    nc.tensor.matmul(pden[:], ones_dr[:],
                                     expT[:, 2 * t2:2 * t2 + 2, ts(sc, NCS)],
                                     start=(t2 == 0), stop=(t2 == ST2 - 1),
                                     perf_mode=DR)
                rb = p2.tile([P, NCS], dt.float32, tag="rb", name=f"rb_{h}_{sc}")
                nc.vector.reciprocal_approx_fast(out=rb[:], in_=pden[:])
                po = p2b.tile([P, NCS], dt.float32, tag="po", name=f"po_{h}_{sc}")
                for t2 in range(ST2):
                    nc.tensor.matmul(po[:], vT[:, 2 * t2:2 * t2 + 2, :],
                                     expT[:, 2 * t2:2 * t2 + 2, ts(sc, NCS)],
                                     start=(t2 == 0), stop=(t2 == ST2 - 1),
                                     perf_mode=DR)
                nc.vector.tensor_tensor(oT_all[:, h, ts(sc, NCS)], po[:], rb[:], OP.mult)

        prev = None
        for h in range(H):
            qkv = []
            for j, mo in ((0, h), (1, KD + h), (2, 2 * KD + h)):
                if h == 0:
                    wblk = wblk0[j]
                else:
                    wblk = p2w.tile([P, KD2, 2, P], dt.float8e4, tag="wblk", bufs=3)
                    nc.sync.dma_start(out=wblk[:], in_=w_in_d.ap()[mo])
                dest = p2.tile([P, S], dt.bfloat16, tag=f"qkv{j}", name=f"qkv{j}_{h}")
                for sc in range(NOS):
                    ps = p2a.tile([P, NCS], dt.float32, tag="mmps",
                                  name=f"qkvps_{h}_{j}_{sc}")
                    for ko2 in range(KD2):
                        nc.tensor.matmul(ps[:], wblk[:, ko2],
                                         x2T[:, 2 * ko2:2 * ko2 + 2, ts(sc, NCS)],
                                         start=(ko2 == 0), stop=(ko2 == KD2 - 1),
                                         perf_mode=DR)
                    nc.scalar.activation(dest[:, ts(sc, NCS)], ps[:], AF.Identity,
                                         bias=bin_sb[:, mo:mo + 1], scale=1.0 / sw_in)
                qkv.append(dest)
            q, k, v = qkv
            # scores^T -> exp (shifted by -ln4 so fp8 e4m3 never overflows)
            expT = p2.tile([P, ST, S], dt.float8e4, tag="expT", name=f"expT_{h}")
            for tt in range(ST):
                for sc in range(NOS):
                    ps = p2b.tile([P, NCS], dt.float32, tag="scps", name=f"scps_{h}_{tt}_{sc}")
                    nc.tensor.matmul(ps[:], k[:, ts(tt, P)], q[:, ts(sc, NCS)],
                                     start=True, stop=True)
                    nc.scalar.activation(expT[:, tt, ts(sc, NCS)], ps[:], AF.Exp,
                                         scale=inv_sqrt_dh, bias=expb_sb[:])
            # v^T via PE transpose (bf16), fp8 conversion on the copy out
            vT = p2.tile([P, ST, P], dt.float8e4, tag="vT", name=f"vT_{h}")
            for tt in range(ST):
                pt = p2c.tile([P, P], dt.bfloat16, tag="vtp", bufs=1,
                              name=f"vtp_{h}_{tt}")
                nc.tensor.transpose(pt[:], v[:, ts(tt, P)], ident[:])
                nc.vector.tensor_copy(vT[:, tt, :], pt[:])
            if prev is not None:
                attn_tail(*prev)
            prev = (h, expT, vT)
        attn_tail(*prev)
        p2c.release()
        p2b.release()
        p2a.release()
        p2.release()
        p2w.release()
        xp.release()  # x2T dead

        # ===== phase 3: out_proj + residual + rmsnorm2 + quant =====
        # x tiles stay in SBUF; wo re-streamed once. x2qT holds the int8
        # activations PRE-SCALED by c1 (bf16), so ff1 needs no c1 broadcast.
        qp = tc.alloc_tile_pool(name="x2qT_pool", bufs=1)
        x2qT = qp.tile([P, KD, S], dt.bfloat16)

        p3c = tc.alloc_tile_pool(name="p3c", bufs=1)
        p3x = tc.alloc_tile_pool(name="p3x", bufs=1)
        p3 = tc.alloc_tile_pool(name="p3", bufs=3)
        p3ps = tc.alloc_tile_pool(name="p3ps", bufs=1, space="PSUM")
        n2w_sb = p3c.tile([P, D], dt.float32)
        nc.sync.dma_start(out=n2w_sb[:], in_=n2w_d.ap())
        b2_sb = p3c.tile([P, D], dt.float32)
        nc.sync.dma_start(out=b2_sb[:], in_=b2_d.ap())

        xg = [p3x.tile([P, D], dt.float32, tag=f"xg{i}", name=f"xg_{i}")
              for i in range(ST)]
        for mt in range(ST):  # seed x tiles with the residual; gpsimd's DMA
            # queue is idle so these 8 MB prefetch during attention instead of
            # blocking out_proj's weight stream on the sync queue
            nc.gpsimd.dma_start(out=xg[mt][:], in_=srcb_d.ap()[ts(mt, P), :])
        # --- out_proj (DoubleRow) in two mt-groups; each group's rmsnorm/quant
        # chain (DVE/ACT) overlaps the next group's / ff1's PE work. wo is
        # streamed once per group. ---
        p3q = tc.alloc_tile_pool(name="p3q", bufs=2)
        p3qps = tc.alloc_tile_pool(name="p3qps", bufs=2, space="PSUM")
        GS = ST // 2

        xq_tiles = {}

        def quant_chain(mt):
            # DVE/ACT-only part (no PE instructions) — overlaps the next
            # group's out_proj matmuls
            xt = xg[mt]
            sq = p3q.tile([P, D], dt.float32, tag="scr", bufs=3, name=f"sq3_{mt}")
            ss = p3q.tile([P, 1], dt.float32, tag="ss", bufs=4)
            nc.scalar.activation(sq[:], xt[:], AF.Square, accum_out=ss[:])
            ms = p3q.tile([P, 1], dt.float32, tag="ms", bufs=4)
            nc.vector.tensor_scalar(ms[:], ss[:], 1.0 / D, EPS, op0=OP.mult, op1=OP.add)
            rt = p3q.tile([P, 1], dt.float32, tag="rt", bufs=4)
            nc.scalar.activation(rt[:], ms[:], AF.Sqrt)
            rs = p3q.tile([P, 1], dt.float32, tag="rs", bufs=4)
            nc.vector.reciprocal(rs[:], rt[:])
            x2 = p3q.tile([P, D], dt.float32, tag="x2", bufs=2)
            nc.vector.scalar_tensor_tensor(x2[:], xt[:], rs[:], n2w_sb[:],
                                           op0=OP.mult, op1=OP.mult)
            # per-token absmax -> scales
            mx = p3q.tile([P, 1], dt.float32, tag="mx", bufs=4)
            nc.vector.tensor_reduce(mx[:], x2[:], axis=mybir.AxisListType.X,
                                    op=OP.max, apply_absolute_value=True)
            mcl = p3q.tile([P, 1], dt.float32, tag="mcl", bufs=4)
            nc.vector.tensor_scalar(mcl[:], mx[:], 1e-5, None, op0=OP.max)
            rc = p3q.tile([P, 1], dt.float32, tag="rc", bufs=4)
            nc.vector.reciprocal(rc[:], mcl[:])
            sc1 = p3q.tile([P, 1], dt.float32, tag="sc1", bufs=4)
            nc.vector.tensor_scalar(sc1[:], rc[:], 127.0, None, op0=OP.mult)
            c1c = p3q.tile([P, 1], dt.float32, tag="c1c", bufs=4)
            nc.vector.tensor_scalar(c1c[:], mcl[:], ws1 / 127.0, None, op0=OP.mult)
            # xq = round(x2 * sc1) * c1c: exact round via fp32 magic, then the
            # dequant scale folded in (bf16 rounding only)
            t1 = p3q.tile([P, D], dt.float32, tag="scr", bufs=3, name=f"t1_{mt}")
            nc.vector.tensor_scalar(t1[:], x2[:], sc1[:], MAGIC, op0=OP.mult, op1=OP.add)
            xq = p3q.tile([P, D], dt.bfloat16, tag="xq", bufs=4, name=f"xq_{mt}")
            nc.vector.tensor_scalar(xq[:], t1[:], MAGIC, c1c[:], op0=OP.subtract, op1=OP.mult)
            xq_tiles[mt] = xq

        def quant_transpose(mt):
            xq = xq_tiles[mt]
            for ko in range(KD):
                pt = p3qps.tile([P, P], dt.bfloat16, tag="tp")
                nc.tensor.transpose(pt[:], xq[:, ts(ko, P)], ident[:])
                nc.vector.tensor_copy(x2qT[:, ko, ts(mt, P)], pt[:])

        for g in range(2):
            mts = list(range(GS * g, GS * (g + 1)))
            for no in range(NOD):
                psy = [p3ps.tile([P, NCD], dt.float32, tag=f"y{i}",
                                 name=f"psy3_{no}_{mt}")
                       for i, mt in enumerate(mts)]
                for ko2 in range(KD2):
                    wch = p3.tile([P, 2, NCD], dt.float8e4, tag="wch", bufs=6)
                    nc.sync.dma_start(out=wch[:], in_=wo_d.ap()[no, ko2])
                    for i, mt in enumerate(mts):
                        nc.tensor.matmul(psy[i][:], oT_all[:, 2 * ko2:2 * ko2 + 2, ts(mt, P)],
                                         wch[:], start=(ko2 == 0), stop=(ko2 == KD2 - 1),
                                         perf_mode=DR)
                for i, mt in enumerate(mts):
                    ch = ts(no, NCD)
                    nc.vector.scalar_tensor_tensor(xg[mt][:, ch], psy[i][:], 1.0 / swo,
                                                   xg[mt][:, ch], op0=OP.mult, op1=OP.add)
                    # spill x + b2 for the ff2 residual here (PE-bound phase)
                    # instead of on the rmsnorm/quant critical chain
                    xo = p3.tile([P, NCD], dt.float32, tag="xo", bufs=4,
                                 name=f"xo_{no}_{mt}")
                    nc.vector.tensor_tensor(xo[:], xg[mt][:, ch], b2_sb[:, ch], OP.add)
                    nc.gpsimd.dma_start(out=xb2_d.ap()[mt][:, ch], in_=xo[:])
            if g == 1:
                for mt in range(GS):  # group 0's PE transposes after g1 out_proj
                    quant_transpose(mt)
            for mt in mts:
                quant_chain(mt)
        for mt in range(GS, ST):
            quant_transpose(mt)
        op_.release()  # oT_all dead
        p3qps.release()
        p3q.release()
        p3ps.release()
        p3.release()
        p3x.release()
        p3c.release()

        # ====== phase 4: ff1 + snake + h2 spill + absmax ======
        # PSUM evictions go straight to ACT (c1/ws1 are folded into x2qT).
        # NOTE: ACT Sin has no range reduction (garbage beyond |x|~2pi), so
        # keep the argument at a*h (not the half-angle 2a*h cos identity).
        p4 = tc.alloc_tile_pool(name="p4", bufs=3)
        p4m = tc.alloc_tile_pool(name="p4m", bufs=1)
        p4ps = tc.alloc_tile_pool(name="p4ps", bufs=3, space="PSUM")
        M_acc = p4m.tile([P, S], dt.float32)
        nc.any.memset(M_acc[:], 0.0)
        for mo in range(KF):
            wblk = p4.tile([P, KD, P], dt.bfloat16, tag="wblk")
            nc.sync.dma_start(out=wblk[:], in_=w1_d.ap()[mo])
            for sc in range(NOS):
                ph = p4ps.tile([P, NCS], dt.float32, tag="ph", name=f"ph_{mo}_{sc}")
                for ko in range(KD):
                    nc.tensor.matmul(ph[:], wblk[:, ko, :], x2qT[:, ko, ts(sc, NCS)],
                                     start=(ko == 0), stop=(ko == KD - 1))
                ch = ts(sc, NCS)
                s_ = p4.tile([P, NCS], dt.float32, tag="s_", name=f"s_{mo}_{sc}")
                nc.scalar.activation(s_[:], ph[:], AF.Sin,
                                     scale=a1_sb[:, mo:mo + 1], bias=ab1_sb[:, mo:mo + 1])
                h_ = p4.tile([P, NCS], dt.float32, tag="h_", name=f"h_{mo}_{sc}")
                nc.scalar.activation(h_[:], ph[:], AF.Identity, bias=b1_sb[:, mo:mo + 1])
                sq_ = p4.tile([P, NCS], dt.float32, tag="sq_", name=f"sq_{mo}_{sc}")
                nc.scalar.activation(sq_[:], s_[:], AF.Square)
                h2_ = p4.tile([P, NCS], dt.float32, tag="h2_", name=f"h2_{mo}_{sc}")
                nc.vector.scalar_tensor_tensor(h2_[:], sq_[:], gam_sb[:, mo:mo + 1], h_[:],
                                               op0=OP.mult, op1=OP.add)
                nc.gpsimd.dma_start(out=h2_d.ap()[mo][:, ch], in_=h2_[:])
                am_ = p4.tile([P, NCS], dt.float32, tag="am_", name=f"am_{mo}_{sc}")
                nc.vector.scalar_tensor_tensor(am_[:], h2_[:], -1.0, h2_[:],
                                               op0=OP.mult, op1=OP.max)
                nc.vector.tensor_tensor(M_acc[:, ch], M_acc[:, ch], am_[:], OP.max)
        # cross-partition absmax via PE transpose + free-axis reduce
        m_tok = p4m.tile([P, ST], dt.float32)
        for c in range(ST):
            pmt = p4ps.tile([P, P], dt.float32, tag="pmt", bufs=2, name=f"pmt_{c}")
            nc.tensor.transpose(pmt[:], M_acc[:, ts(c, P)], identf[:])
            nc.vector.tensor_reduce(m_tok[:, c:c + 1], pmt[:], axis=mybir.AxisListType.X, op=OP.max)
        mcl2 = p4m.tile([P, ST], dt.float32)
        nc.vector.tensor_scalar(mcl2[:], m_tok[:], 1e-5, None, op0=OP.max)
        rc2 = p4m.tile([P, ST], dt.float32)
        nc.vector.reciprocal(rc2[:], mcl2[:])
        sc2_tok = p4m.tile([P, ST], dt.float32)
        nc.vector.tensor_scalar(sc2_tok[:], rc2[:], 127.0, None, op0=OP.mult)
        nc.vector.tensor_scalar(c2_tok[:], mcl2[:], ws2 / 127.0, None, op0=OP.mult)
        sc2row = p4m.tile([1, S], dt.float32)
        for mt in range(ST):
            nc.sync.dma_start(out=sc2row[0:1, ts(mt, P)], in_=sc2_tok[:, mt:mt + 1])
        for sc in range(NOS):
            pb = p4ps.tile([P, NCS], dt.float32, tag="pb", bufs=2, name=f"pb4_{sc}")
            nc.tensor.matmul(pb[:], ones_1[:], sc2row[:, ts(sc, NCS)], start=True, stop=True)
            nc.vector.tensor_copy(sc2_bc[:, ts(sc, NCS)], pb[:])
        p4ps.release()
        p4m.release()
        p4.release()
        qp.release()  # x2qT dead

        # ===== phase 5+6: ff2, h2 quantization fused into the first no pass =====
        # quant DVE work alternates vector/gpsimd per ko so the first pass is
        # not rate-limited by one engine
        q2p = tc.alloc_tile_pool(name="xq2_pool", bufs=1)
        xq2 = q2p.tile([P, KF, S], dt.bfloat16)
        p6 = tc.alloc_tile_pool(name="p6", bufs=3)
        p6ps = tc.alloc_tile_pool(name="p6ps", bufs=1, space="PSUM")
        for no in range(NOD):
            psy = [p6ps.tile([P, NCD], dt.float32, tag=f"y{mt}", name=f"psy6_{no}_{mt}")
                   for mt in range(ST)]
            xchs = []
            for mt in range(ST):  # prefetch residual chunks
                xch = p6.tile([P, NCD], dt.float32, tag="xch", bufs=ST,
                              name=f"xch6_{no}_{mt}")
                nc.gpsimd.dma_start(out=xch[:], in_=xb2_d.ap()[mt][:, ts(no, NCD)])
                xchs.append(xch)
            for ko in range(KF):
                if no == 0:
                    # quantize h2[ko] -> exact ints in bf16, just ahead of first
                    # use. Only the scale-mult runs on DVE; the magic round is
                    # two ACT adds (exact fp32) so DVE doesn't rate-limit the
                    # first no pass.
                    h2t = p6.tile([P, S], dt.float32, tag="h2t", bufs=3)
                    nc.sync.dma_start(out=h2t[:], in_=h2_d.ap()[ko])
                    m1 = p6.tile([P, S], dt.float32, tag="m1", bufs=2)
                    nc.vector.tensor_tensor(m1[:], h2t[:], sc2_bc[:], OP.mult)
                    r1 = p6.tile([P, S], dt.float32, tag="r1", bufs=2)
                    nc.scalar.activation(r1[:], m1[:], AF.Identity, bias=magicp[:])
                    nc.scalar.activation(xq2[:, ko, :], r1[:], AF.Identity,
                                         bias=magicn[:])
                wch = p6.tile([P, NCD], dt.bfloat16, tag="wch", bufs=4)
                nc.sync.dma_start(out=wch[:], in_=w2_d.ap()[no, ko])
                for mt in range(ST):
                    nc.tensor.matmul(psy[mt][:], xq2[:, ko, ts(mt, P)], wch[:],
                                     start=(ko == 0), stop=(ko == KF - 1))
            for mt in range(ST):
                # two-stage evict: ACT scales + frees the PSUM bank fast,
                # DVE adds the residual off the critical path
                oe1 = p6.tile([P, NCD], dt.float32, tag="oe1", bufs=2, name=f"oe1_{no}_{mt}")
                nc.scalar.activation(oe1[:], psy[mt][:], AF.Identity,
                                     scale=c2_tok[:, mt:mt + 1])
                oe = p6.tile([P, NCD], dt.float32, tag="oe", bufs=2, name=f"oe_{no}_{mt}")
                nc.vector.tensor_tensor(oe[:], oe1[:], xchs[mt][:], OP.add)
                nc.sync.dma_start(out=out_d.ap()[ts(mt, P), ts(no, NCD)], in_=oe[:])
        p6ps.release()
        p6.release()
        q2p.release()
        cp.release()
    return nc


# ---------------------------------------------------------------- driver

def _get_compiled(key, S, D, H, DFF, ws1, ws2, sw_in, swo):
    if key in _CACHE:
        return _CACHE[key]
    from concourse import bacc

    nc = bacc.Bacc("TRN2", target_bir_lowering=False, debug=False, num_devices=NCORES)
    build_program(nc, S=S, D=D, H=H, DFF=DFF, ws1=ws1, ws2=ws2, sw_in=sw_in, swo=swo)
    nc.compile()
    _CACHE[key] = nc
    return nc


def make_in_maps(inputs):
    src = np.asarray(inputs["src"], dtype=np.float32)
    B, S, D = src.shape
    H = H_FULL
    DFF = inputs["ff1_w"].shape[0]
    arrays, ws1, ws2, sw_in, swo = _prep_arrays(inputs, S, D, H, DFF)
    srcb = src + np.asarray(inputs["out_proj_b"], dtype=np.float32)[None, None, :]
    in_maps = []
    for c in range(NCORES):
        m = dict(arrays)
        m["src"] = np.ascontiguousarray(src[c])
        m["srcb"] = np.ascontiguousarray(srcb[c])
        in_maps.append(m)
    return in_maps, (S, D, H, DFF, ws1, ws2, sw_in, swo)


def kernel(**inputs):
    from concourse.bass_utils import run_bass_kernel_spmd

    in_maps, (S, D, H, DFF, ws1, ws2, sw_in, swo) = make_in_maps(inputs)
    assert np.asarray(inputs["src"]).shape[0] == NCORES
    nc = _get_compiled(("full", S, D, H, DFF, ws1, ws2, sw_in, swo),
                       S, D, H, DFF, ws1, ws2, sw_in, swo)
    res = run_bass_kernel_spmd(nc, in_maps, core_ids=list(range(NCORES)))
    out = np.stack([res.results[c]["out"] for c in range(NCORES)], axis=0)
    return out.astype(np.float32)
